# revision 30
# baseline (speedup 1.0000x reference)
"""Trainium2 Bass kernel for CausalAttention (sliding-window + scale-frame sparse attention).

Problem shape (hardcoded): B=1, N=4096, C=512, H=8, Dh=64, frame_seqlen=256,
sliding_window_size=2, num_frame_per_block=1, num_frame_for_scale=2.

Sharding: sequence-parallel over 8 NeuronCores. Core i owns queries
[512*i, 512*(i+1)) (= frames 2i, 2i+1) and returns that slice of the final
output. Keys needed per core: the 512 "scale" tokens (frames 0,1; attended by
every query unconditionally per the reference mask) plus a 3-frame window
{2i-1, 2i, 2i+1} (768 tokens). No collectives; host concatenates the slices.

Per-core device pipeline (all matmuls bf16 with fp32 PSUM accumulation):
  1. QKV projection in transposed layout: QT/KT = W @ x^T (channels on
     partitions), V in natural [token, dh] layout with a ones-column per head
     appended (so the attention-value matmul also produces softmax sums).
  2. Scores computed transposed, S^T[k, q] = K @ Q^T, per head, into merged
     2-bank PSUM tiles (fewer, larger exp activations on ScalarE).
  3. exp on ScalarE straight out of PSUM (softmax scale folded into the
     activation's `scale`; no max-subtraction needed: scores are O(10) so
     fp32 exp cannot overflow; this matches jax softmax to rounding error).
  4. Mask structure applied multiplicatively to the bf16 probabilities in a
     single [128, 2048] DVE multiply per head (whole-block validity flags and
     tril for the diagonal frame, all baked into one per-core mask image).
  5. O^T = V'^T @ P^T accumulated over key tiles; row 64 of the accumulator
     holds the softmax denominators; normalize via a [128, 4]-reshaped DVE
     reciprocal and a broadcast DMA (sync engine), software-pipelined one
     head behind so no engine stalls on the chain.
  6. out^T accumulated head-pair-packed (contract 128): 16 matmuls at the
     tail, overlapping the last head's normalize chain; bf16 output DMA.
"""

from contextlib import ExitStack

import numpy as np
import ml_dtypes

N, C, H, DH = 4096, 512, 8, 64
F = 256                 # frame_seqlen
NCORES = 8
NQ = N // NCORES        # 512 queries per core (2 frames)
KS = 512                # scale tokens (frames 0,1)
KW = 3 * F              # window tokens per core
NK = KS + KW            # 1280 keys per core
BF16 = ml_dtypes.bfloat16

_CACHE = {}


def _build(repeat=1):
    """Build + compile the (single, SPMD) Bass program. Returns nc."""
    import concourse.bass as bass  # noqa: F401
    import concourse.mybir as mybir
    import concourse.tile as tile
    from concourse import bacc

    f32 = mybir.dt.float32
    bf16 = mybir.dt.bfloat16
    EXP = mybir.ActivationFunctionType.Exp
    CPY = mybir.ActivationFunctionType.Identity

    nc = bacc.Bacc("TRN2", target_bir_lowering=False, debug=False)

    xT = nc.dram_tensor("xT", [C, NK], bf16, kind="ExternalInput")
    wqT = nc.dram_tensor("wqT", [C, C], bf16, kind="ExternalInput")
    wkT = nc.dram_tensor("wkT", [C, C], bf16, kind="ExternalInput")
    wvT = nc.dram_tensor("wvT", [C, C], bf16, kind="ExternalInput")
    wp2 = nc.dram_tensor("wp2", [128, 4 * C], bf16, kind="ExternalInput")
    btab = nc.dram_tensor("btab", [128, 12], f32, kind="ExternalInput")
    dmsk = nc.dram_tensor("dmsk", [128, 4 * NQ], bf16, kind="ExternalInput")
    outT = nc.dram_tensor("outT", [C, NQ], bf16, kind="ExternalOutput")

    with tile.TileContext(nc) as tc, ExitStack() as ctx:
        cp = ctx.enter_context(tc.tile_pool(name="const", bufs=1))
        dp = cp
        ptp = ctx.enter_context(tc.tile_pool(name="pt", bufs=5))
        recp = ptp
        psp = ctx.enter_context(tc.tile_pool(name="ps", bufs=2, space="PSUM"))
        pap = ctx.enter_context(tc.tile_pool(name="pa", bufs=4, space="PSUM"))

        def body():
            xs = cp.tile([128, 4, NK], bf16, tag="xs")
            wq = cp.tile([128, 4, C], bf16, tag="wq")
            wk = cp.tile([128, 4, C], bf16, tag="wk")
            wv = cp.tile([128, 4, C], bf16, tag="wv")
            xr = xT.ap().rearrange("(a p) t -> p a t", p=128)
            wqr = wqT.ap().rearrange("(a p) o -> p a o", p=128)
            wkr = wkT.ap().rearrange("(a p) o -> p a o", p=128)
            wvr = wvT.ap().rearrange("(a p) o -> p a o", p=128)
            # chunked input DMAs (4 descriptors/tensor -> parallel DMA
            # queues). Critical path (wv+xs for the V projection, then
            # wq/wk for QKT) leads; bulky non-critical tensors (mask image,
            # proj weight) are sequenced after on the sync queue so their
            # transfers don't steal HBM bandwidth from xs.
            for ci in range(4):
                nc.sync.dma_start(xs[:, ci, :], xr[:, ci, :])
            bt = cp.tile([128, 12], f32, tag="bt")
            nc.gpsimd.dma_start(bt[:], btab.ap())
            for ci in range(4):
                nc.scalar.dma_start(wq[:, ci, :], wqr[:, ci, :])
                nc.gpsimd.dma_start(wk[:, ci, :], wkr[:, ci, :])
            for ci in range(4):
                nc.sync.dma_start(wv[:, ci, :], wvr[:, ci, :])
            dm = cp.tile([128, 4, NQ], bf16, tag="dm")
            for a in range(2):
                nc.sync.dma_start(
                    dm[:, 2 * a:2 * a + 2, :],
                    dmsk.ap().rearrange("p (a q) -> p a q", a=4)[:, 2 * a:2 * a + 2, :])
            wp = cp.tile([128, 4, C], bf16, tag="wp")
            nc.sync.dma_start(wp[:], wp2.ap().rearrange("p (a o) -> p a o", a=4))

            # warmup matmuls: overlap the input DMA phase, get HAM to K=8/8
            wmup = cp.tile([128, 512], bf16, tag="wmup")
            nc.vector.memset(wmup[:], 0.0)
            ones64f = cp.tile([1, 64], f32, tag="ones64f")
            nc.vector.memset(ones64f[:], 1.0)
            wps = pap.tile([128, 512], f32, tag="pa", name="wps")
            for _ in range(14):
                nc.tensor.matmul(wps[:], lhsT=wmup[:, 0:128], rhs=wmup[:],
                                 start=True, stop=True, skip_group_check=True)
            dumt = cp.tile([1, 16], bf16, tag="dumt")
            nc.scalar.activation(dumt[:], wmup[0:1, 0:16], EXP, scale=1.0)

            QT = dp.tile([128, 4, NQ], bf16, tag="QT")
            KT = dp.tile([128, 4, NK], bf16, tag="KT")
            V = dp.tile([128, 10, H, DH + 1], bf16, tag="V")
            OT2 = dp.tile([128, 4, NQ], bf16, tag="OT2")
            oT = dp.tile([128, 4, NQ], bf16, tag="oT")

            # QKT psums rotate through the score pool; V-projection psums
            # ping-pong the pa pool (free until the first AV accumulator)
            _qk = [0]

            def qkv_psum():
                _qk[0] += 1
                return pap.tile([128, 512], f32, tag="pa", name="qps")

            # ---- V projection (natural layout) + ones column; groups are
            # emitted interleaved between the first S-blocks (PE slack under
            # the exp-bound pipeline), psums from the pa pool, copies on DVE
            nc.vector.memset(V[:, :, :, DH:DH + 1], 1.0)

            def v_groups(t0, t1):
                for tt in range(t0, t1):
                    ps = pap.tile([128, 512], f32, tag="pa", name="vps")
                    for ci in range(4):
                        nc.tensor.matmul(ps[:], lhsT=xs[:, ci, 128 * tt:128 * (tt + 1)],
                                         rhs=wv[:, ci, :], start=(ci == 0), stop=(ci == 3))
                    nc.vector.tensor_copy(V[:, tt, :, 0:DH],
                                          ps[:].rearrange("p (h d) -> p h d", h=H))

            # ---- normalize chain (for head h), emitted one head late ----
            sden = {}

            def norm_chain_a(h):
                sm, av = sden.pop(h)
                nc.vector.tensor_copy(sm[:], av[0:65, :])
                rs = recp.tile([128, 4], f32, tag="rs", name="rs")
                nc.sync.dma_start(rs[:], sm[64:65, :])
                nc.vector.reciprocal(rs[:], rs[:])
                rcb = recp.tile([64, NQ], f32, tag="rcb", name="rcb")
                nc.sync.dma_start(rcb[0:1, :], rs[:])
                nc.gpsimd.partition_broadcast(rcb[:, :], rcb[0:1, :])
                sden[h] = (sm, rcb)

            def norm_chain_b(h):
                po, prow = h // 2, slice((h % 2) * 64, (h % 2) * 64 + 64)
                sm, rcb = sden.pop(h)
                nc.vector.tensor_mul(OT2[prow, po, :], sm[0:64, :], rcb[:])

            # ---- per-pair Q^T/K^T projection interleaved with attention:
            # pair p+1's projections are emitted between heads 2p and 2p+1 so
            # ScalarE never runs dry of exps at pair boundaries
            def qkt_proj(p):
                ps = qkv_psum()
                for ci in range(4):
                    nc.tensor.matmul(ps[:], lhsT=wq[:, ci, 128 * p:128 * (p + 1)],
                                     rhs=xs[:, ci, KS + F:KS + F + NQ],
                                     start=(ci == 0), stop=(ci == 3))
                nc.vector.tensor_scalar_add(QT[:, p, :], ps[:], bt[:, p:p + 1])
                for t0, t1 in ((0, 512), (512, 1024), (1024, 1280)):
                    ps = qkv_psum()
                    for ci in range(4):
                        nc.tensor.matmul(ps[:, 0:t1 - t0],
                                         lhsT=wk[:, ci, 128 * p:128 * (p + 1)],
                                         rhs=xs[:, ci, t0:t1],
                                         start=(ci == 0), stop=(ci == 3))
                    nc.vector.tensor_scalar_add(KT[:, p, t0:t1], ps[:, 0:t1 - t0],
                                                bt[:, 4 + p:5 + p])

            qkt_proj(0)
            pend = {}

            def s_block(h):
                po = h // 2
                prow = slice((h % 2) * 64, (h % 2) * 64 + 64)
                qh = QT[prow, po, :]

                def kslice(kt):
                    return KT[prow, po, 128 * kt:128 * (kt + 1)]

                # scale keys (k-tiles 0..3): full query range, no mask.
                # two 2-bank psum tiles -> two big exps
                pts = ptp.tile([128, 4, NQ], bf16, tag="pts", name="pts")
                for g in range(2):
                    sc = psp.tile([128, 2, 512], f32, tag="ps", name="sc")
                    for j in range(2):
                        nc.tensor.matmul(sc[:, j, :], lhsT=kslice(2 * g + j),
                                         rhs=qh, start=True, stop=True)
                    nc.scalar.activation(pts[:, 2 * g:2 * g + 2, :], sc[:],
                                         EXP, scale=float(DH) ** -0.5)
                # window keys, merged layout [128, 4, 512]:
                #  slot0: kt4 @ q0:256 | kt5 @ q0:256
                #  slot1: kt6 @ q0:512
                #  slot2: kt7 @ q0:512 (q0:128 is masked to zero)
                #  slot3: kt8 @ q256:512 | kt9 @ q256:512
                pw = ptp.tile([128, 4, NQ], bf16, tag="pw", name="pw")
                wn = psp.tile([128, 2, 512], f32, tag="ps", name="wn")
                nc.tensor.matmul(wn[:, 0, 0:256], lhsT=kslice(4),
                                 rhs=qh[:, 0:256], start=True, stop=True)
                nc.tensor.matmul(wn[:, 0, 256:512], lhsT=kslice(5),
                                 rhs=qh[:, 0:256], start=True, stop=True)
                nc.tensor.matmul(wn[:, 1, :], lhsT=kslice(6),
                                 rhs=qh, start=True, stop=True)
                nc.scalar.activation(pw[:, 0:2, :], wn[:],
                                     EXP, scale=float(DH) ** -0.5)
                wn = psp.tile([128, 2, 512], f32, tag="ps", name="wn")
                nc.tensor.matmul(wn[:, 0, :], lhsT=kslice(7),
                                 rhs=qh, start=True, stop=True)
                nc.tensor.matmul(wn[:, 1, 0:256], lhsT=kslice(8),
                                 rhs=qh[:, 256:512], start=True, stop=True)
                nc.tensor.matmul(wn[:, 1, 256:512], lhsT=kslice(9),
                                 rhs=qh[:, 256:512], start=True, stop=True)
                nc.scalar.activation(pw[:, 2:4, :], wn[:],
                                     EXP, scale=float(DH) ** -0.5)
                pend[h] = (pts, pw)

            def av_block(h):
                pts, pw = pend.pop(h)
                nc.vector.tensor_mul(pw[:], pw[:], dm[:])
                # O^T accumulation (+ sums in row 64 via the ones column)
                av = pap.tile([128, 512], f32, tag="pa", name="av")
                for kt in range(4):
                    nc.tensor.matmul(av[0:65, :], lhsT=V[:, kt, h, :],
                                     rhs=pts[:, kt, :],
                                     start=(kt == 0), stop=False,
                                     skip_group_check=True)
                nc.tensor.matmul(av[0:65, :], lhsT=V[:, 6, h, :],
                                 rhs=pw[:, 1, :],
                                 start=False, stop=False,
                                 skip_group_check=True)
                nc.tensor.matmul(av[0:65, :], lhsT=V[:, 7, h, :],
                                 rhs=pw[:, 2, :],
                                 start=False, stop=False,
                                 skip_group_check=True)
                for j, kt in enumerate((4, 5)):
                    nc.tensor.matmul(av[0:65, 0:256], lhsT=V[:, kt, h, :],
                                     rhs=pw[:, 0, 256 * j:256 * (j + 1)],
                                     start=False, stop=(j == 1),
                                     skip_group_check=True)
                for j, kt in enumerate((8, 9)):
                    nc.tensor.matmul(av[0:65, 256:512], lhsT=V[:, kt, h, :],
                                     rhs=pw[:, 3, 256 * j:256 * (j + 1)],
                                     start=False, stop=(j == 1),
                                     skip_group_check=True)
                sm = recp.tile([65, NQ], f32, tag="sm", name="sm")
                sden[h] = (sm, av)

            # ---- drive the 8 heads. Prologue: the first four S-blocks with
            # the V-projection groups interleaved into the PE slack of the
            # exp-bound pipeline; then a steady loop with 4-head lookahead.
            s_block(0)
            s_block(1)
            v_groups(0, 4)
            qkt_proj(1)
            s_block(2)
            v_groups(4, 10)
            for h in range(8):
                if h > 0:
                    norm_chain_a(h - 1)
                nxt = h + 3
                if nxt < 8:
                    if nxt % 2 == 0:
                        qkt_proj(nxt // 2)
                    s_block(nxt)
                av_block(h)
                if h > 0:
                    norm_chain_b(h - 1)

            # ---- output projection: head-pair-packed, contract 128 ----
            od = outT.ap().rearrange("(a p) q -> p a q", p=128)
            pjs = []
            for ot in range(4):
                if ot % 2 == 0:
                    pj = pap.tile([128, 512], f32, tag="pa", name="pj")
                else:
                    pj = psp.tile([128, 2, 512], f32, tag="ps", name="pj")[:, 0, :]
                pjs.append(pj)
            # pairs 0..2 depend on already-finished OT2 columns; emit them
            # first so the PE keeps streaming while head 7's chain completes.
            for pr in range(3):
                for ot in range(4):
                    nc.tensor.matmul(pjs[ot][:],
                                     lhsT=wp[:, pr, 128 * ot:128 * (ot + 1)],
                                     rhs=OT2[:, pr, :], start=(pr == 0),
                                     stop=False, skip_group_check=True)
            # head 7's chain with a PE rank-1 broadcast (keeps the PE warm and
            # avoids the gpsimd queue latency right at the tail)
            sm7, av7 = sden.pop(7)
            nc.vector.tensor_copy(sm7[:], av7[0:65, :])
            rs7 = recp.tile([128, 4], f32, tag="rs", name="rs")
            nc.sync.dma_start(rs7[:], sm7[64:65, :])
            nc.vector.reciprocal(rs7[:], rs7[:])
            rr7 = recp.tile([1, NQ], f32, tag="rr7", name="rr7")
            nc.sync.dma_start(rr7[:], rs7[:])
            bp7 = pap.tile([128, 512], f32, tag="pa", name="bp7")
            nc.tensor.matmul(bp7[0:64, :], lhsT=ones64f[0:1, :], rhs=rr7[0:1, :],
                             start=True, stop=True, skip_group_check=True)
            nc.vector.tensor_mul(OT2[64:128, 3, :], sm7[0:64, :], bp7[0:64, :])
            for ot in range(4):
                nc.tensor.matmul(pjs[ot][:],
                                 lhsT=wp[:, 3, 128 * ot:128 * (ot + 1)],
                                 rhs=OT2[:, 3, :], start=False,
                                 stop=True, skip_group_check=True)
            for ot in range(4):
                if ot % 2 == 0:
                    nc.scalar.activation(oT[:, ot, :], pjs[ot][:], CPY,
                                         bias=bt[:, 8 + ot:9 + ot], scale=1.0)
                else:
                    nc.vector.tensor_scalar_add(oT[:, ot, :], pjs[ot][:],
                                                bt[:, 8 + ot:9 + ot])
                eng = (nc.sync, nc.gpsimd, nc.scalar, nc.sync)[ot]
                eng.dma_start(od[:, ot, :], oT[:, ot, :])

        if repeat == 1:
            body()
        else:
            with tc.For_i(0, repeat, 1):
                body()

    nc.compile()
    return nc


def _get_nc(repeat=1):
    key = ("nc", repeat)
    if key not in _CACHE:
        _CACHE[key] = _build(repeat)
    return _CACHE[key]


def _host_prep(x, qkv_w, qkv_b, proj_w, proj_b):
    """Build the 8 per-core input maps."""
    x = np.asarray(x, np.float32).reshape(N, C)
    qkv_w = np.asarray(qkv_w, np.float32)
    qkv_b = np.asarray(qkv_b, np.float32)
    proj_w = np.asarray(proj_w, np.float32)
    proj_b = np.asarray(proj_b, np.float32)

    xs_bf = x.astype(BF16)
    xT_scale = np.ascontiguousarray(xs_bf[0:KS].T)            # [C, 512]
    wqT = np.ascontiguousarray(qkv_w[0:C].T.astype(BF16))
    wkT = np.ascontiguousarray(qkv_w[C:2 * C].T.astype(BF16))
    wvT = np.ascontiguousarray(qkv_w[2 * C:3 * C].T.astype(BF16))
    # head-pair-packed proj weight: wp2[dd, pr*C + c] = proj_w[c, 128*pr + dd]
    wp2 = np.ascontiguousarray(
        proj_w.T.reshape(4, 128, C).transpose(1, 0, 2).reshape(128, 4 * C)
        .astype(BF16))

    # value-bias folds through normalized attention into the proj bias:
    # O = sum_k phat_k (V_k + vb) = O_hat + vb, so out += vb @ proj_w.T
    pb_eff = proj_b + qkv_b[2 * C:3 * C] @ proj_w.T
    btab = np.zeros((128, 12), np.float32)
    for ot in range(4):
        btab[:, ot] = qkv_b[0:C][128 * ot:128 * (ot + 1)]
        btab[:, 4 + ot] = qkv_b[C:2 * C][128 * ot:128 * (ot + 1)]
        btab[:, 8 + ot] = pb_eff[128 * ot:128 * (ot + 1)]

    # tril01[j, q] = 1 if key j <= query q (within the same frame)
    tril01 = (np.arange(F)[:, None] <= np.arange(F)[None, :])

    in_maps = []
    for i in range(NCORES):
        win = np.zeros((KW, C), BF16)
        lo = F * (2 * i - 1)
        src = xs_bf[max(0, lo):F * (2 * i + 2)]
        win[KW - len(src):] = src
        xTi = np.empty((C, NK), BF16)
        xTi[:, 0:KS] = xT_scale
        xTi[:, KS:] = win.T

        vf = 1.0 if (2 * i - 1) >= 2 else 0.0
        vd = np.array([1.0 if (2 * i) >= 2 else 0.0,
                       1.0 if (2 * i + 1) >= 2 else 0.0], np.float32)
        # mask image, [128, 4, 512]:
        #  slot0: kt4 flag | kt5 flag (both only live for q0:256)
        #  slot1 (kt6): tril*vd0 for q0:256, vd0 for q256:512
        #  slot2 (kt7): 0 for q0:128, tril*vd0 for q128:256, vd0 for q256:512
        #  slot3 (kt8, kt9): tril halves * vd1 (queries q256:512)
        dmsk = np.zeros((128, 4, NQ), np.float32)
        dmsk[:, 0, :] = vf
        dmsk[:, 1, 0:256] = tril01[0:128, :] * vd[0]
        dmsk[:, 1, 256:512] = vd[0]
        dmsk[:, 2, 0:128] = 0.0
        dmsk[:, 2, 128:256] = tril01[128:256, 128:256] * vd[0]
        dmsk[:, 2, 256:512] = vd[0]
        dmsk[:, 3, 0:256] = tril01[0:128, :] * vd[1]
        dmsk[:, 3, 256:512] = tril01[128:256, :] * vd[1]
        in_maps.append({
            "xT": xTi, "wqT": wqT, "wkT": wkT, "wvT": wvT, "wp2": wp2,
            "btab": btab,
            "dmsk": dmsk.reshape(128, 4 * NQ).astype(BF16),
        })
    return in_maps


def _check_fixed_params(block_mask, video_mask, frame_seqlen,
                        sliding_window_size, num_frame_per_block,
                        num_frame_for_scale):
    if int(frame_seqlen) != F or int(sliding_window_size) != 2 \
            or int(num_frame_per_block) != 1 or int(num_frame_for_scale) != 2:
        return False
    vm = np.asarray(video_mask)
    if not bool(vm.all()):
        return False
    bm = np.asarray(block_mask)
    if bm.shape != (N, N):
        return False
    # spot-check causality structure of block_mask (full check is 16M bools)
    idx = np.linspace(0, N - 1, 64).astype(int)
    sub = bm[np.ix_(idx, idx)]
    if not np.array_equal(sub, np.tril(np.ones_like(sub))):
        return False
    return True


def _numpy_reference(x, block_mask, video_mask, qkv_w, qkv_b, proj_w, proj_b,
                     frame_seqlen, sliding_window_size, num_frame_per_block,
                     num_frame_for_scale):
    """Fallback: direct numpy evaluation of the reference semantics."""
    x = np.asarray(x, np.float32)
    b, n, c = x.shape
    dh = c // H
    qkv = (x @ np.asarray(qkv_w).T + np.asarray(qkv_b)).reshape(b, n, 3, H, dh)
    qkv = qkv.transpose(2, 0, 3, 1, 4)
    q, k, v = qkv[0], qkv[1], qkv[2]
    mask = np.asarray(block_mask)[:n, :n][None, None]
    vm = np.asarray(video_mask)[:, None, None, None]
    mask = mask | ~vm
    fs = int(frame_seqlen)
    if int(sliding_window_size) > 0 and fs is not None:
        f = np.arange(n) // fs
        w = int(sliding_window_size) * int(num_frame_per_block)
        sliding = (f[None, :] <= f[:, None]) & (f[None, :] >= f[:, None] - w + 1)
        mask = mask & sliding[None, None]
        if int(num_frame_for_scale) > 0:
            s = int(num_frame_for_scale) * fs
            mask = mask.copy()
            mask[:, :, :, :s] = True
    scores = np.einsum('bhqd,bhkd->bhqk', q, k) * (dh ** -0.5)
    scores = np.where(mask, scores, np.float32(-1e30))
    scores -= scores.max(axis=-1, keepdims=True)
    e = np.exp(scores)
    attn = e / e.sum(axis=-1, keepdims=True)
    o = np.einsum('bhqk,bhkd->bhqd', attn, v)
    o = o.transpose(0, 2, 1, 3).reshape(b, n, c)
    return (o @ np.asarray(proj_w).T + np.asarray(proj_b)).astype(np.float32)


def kernel(x, block_mask, video_mask, qkv_w, qkv_b, proj_w, proj_b,
           frame_seqlen, sliding_window_size, num_frame_per_block,
           num_frame_for_scale):
    if not _check_fixed_params(block_mask, video_mask, frame_seqlen,
                               sliding_window_size, num_frame_per_block,
                               num_frame_for_scale):
        return _numpy_reference(x, block_mask, video_mask, qkv_w, qkv_b,
                                proj_w, proj_b, frame_seqlen,
                                sliding_window_size, num_frame_per_block,
                                num_frame_for_scale)

    from concourse.bass_utils import run_bass_kernel_spmd

    nc = _get_nc()
    in_maps = _host_prep(x, qkv_w, qkv_b, proj_w, proj_b)
    res = run_bass_kernel_spmd(nc, in_maps, core_ids=list(range(NCORES)))
    out = np.empty((N, C), np.float32)
    for i in range(NCORES):
        out[NQ * i:NQ * (i + 1)] = res.results[i]["outT"].T
    return out.reshape(1, N, C)


# revision 31
# speedup vs baseline: 1.0472x; 1.0472x over previous
"""Trainium2 Bass kernel for CausalAttention (sliding-window + scale-frame sparse attention).

Problem shape (hardcoded): B=1, N=4096, C=512, H=8, Dh=64, frame_seqlen=256,
sliding_window_size=2, num_frame_per_block=1, num_frame_for_scale=2.

Sharding: sequence-parallel over 8 NeuronCores. Core i owns queries
[512*i, 512*(i+1)) (= frames 2i, 2i+1) and returns that slice of the final
output. Keys needed per core: the 512 "scale" tokens (frames 0,1; attended by
every query unconditionally per the reference mask) plus a 3-frame window
{2i-1, 2i, 2i+1} (768 tokens). No collectives; host concatenates the slices.

Per-core device pipeline (all matmuls bf16 with fp32 PSUM accumulation):
  1. QKV projection in transposed layout: QT/KT = W @ x^T (channels on
     partitions), V in natural [token, dh] layout with a ones-column per head
     appended (so the attention-value matmul also produces softmax sums).
  2. Scores computed transposed, S^T[k, q] = K @ Q^T, per head, into merged
     2-bank PSUM tiles (fewer, larger exp activations on ScalarE).
  3. exp on ScalarE straight out of PSUM (softmax scale folded into the
     activation's `scale`; no max-subtraction needed: scores are O(10) so
     fp32 exp cannot overflow; this matches jax softmax to rounding error).
  4. Mask structure applied multiplicatively to the bf16 probabilities in a
     single [128, 2048] DVE multiply per head (whole-block validity flags and
     tril for the diagonal frame, all baked into one per-core mask image).
  5. O^T = V'^T @ P^T accumulated over key tiles; row 64 of the accumulator
     holds the softmax denominators; normalize via a [128, 4]-reshaped DVE
     reciprocal and a broadcast DMA (sync engine), software-pipelined one
     head behind so no engine stalls on the chain.
  6. out^T accumulated head-pair-packed (contract 128): 16 matmuls at the
     tail, overlapping the last head's normalize chain; bf16 output DMA.
"""

from contextlib import ExitStack

import numpy as np
import ml_dtypes

N, C, H, DH = 4096, 512, 8, 64
F = 256                 # frame_seqlen
NCORES = 8
NQ = N // NCORES        # 512 queries per core (2 frames)
KS = 512                # scale tokens (frames 0,1)
KW = 3 * F              # window tokens per core
NK = KS + KW            # 1280 keys per core
BF16 = ml_dtypes.bfloat16

_CACHE = {}


def _build(repeat=1):
    """Build + compile the (single, SPMD) Bass program. Returns nc."""
    import concourse.bass as bass  # noqa: F401
    import concourse.mybir as mybir
    import concourse.tile as tile
    from concourse import bacc

    f32 = mybir.dt.float32
    bf16 = mybir.dt.bfloat16
    EXP = mybir.ActivationFunctionType.Exp
    CPY = mybir.ActivationFunctionType.Identity

    nc = bacc.Bacc("TRN2", target_bir_lowering=False, debug=False)

    xT = nc.dram_tensor("xT", [C, NK], bf16, kind="ExternalInput")
    wqT = nc.dram_tensor("wqT", [C, C], bf16, kind="ExternalInput")
    wkT = nc.dram_tensor("wkT", [C, C], bf16, kind="ExternalInput")
    wvT = nc.dram_tensor("wvT", [C, C], bf16, kind="ExternalInput")
    wp2 = nc.dram_tensor("wp2", [128, 4 * C], bf16, kind="ExternalInput")
    btab = nc.dram_tensor("btab", [128, 12], f32, kind="ExternalInput")
    dmsk = nc.dram_tensor("dmsk", [128, 4 * NQ], bf16, kind="ExternalInput")
    outT = nc.dram_tensor("outT", [C, NQ], bf16, kind="ExternalOutput")

    with tile.TileContext(nc) as tc, ExitStack() as ctx:
        cp = ctx.enter_context(tc.tile_pool(name="const", bufs=1))
        dp = ctx.enter_context(tc.tile_pool(name="data", bufs=1))
        ptp = ctx.enter_context(tc.tile_pool(name="pt", bufs=2))
        recp = ctx.enter_context(tc.tile_pool(name="rec", bufs=2))
        psp = ctx.enter_context(tc.tile_pool(name="ps", bufs=3, space="PSUM"))
        pap = ctx.enter_context(tc.tile_pool(name="pa", bufs=2, space="PSUM"))

        def body():
            xs = cp.tile([128, 4, NK], bf16, tag="xs")
            wq = cp.tile([128, 4, C], bf16, tag="wq")
            wk = cp.tile([128, 4, C], bf16, tag="wk")
            wv = cp.tile([128, 4, C], bf16, tag="wv")
            xr = xT.ap().rearrange("(a p) t -> p a t", p=128)
            wqr = wqT.ap().rearrange("(a p) o -> p a o", p=128)
            wkr = wkT.ap().rearrange("(a p) o -> p a o", p=128)
            wvr = wvT.ap().rearrange("(a p) o -> p a o", p=128)
            # chunked input DMAs (4 descriptors/tensor -> parallel DMA
            # queues). Critical path (wv+xs for the V projection, then
            # wq/wk for QKT) leads; bulky non-critical tensors (mask image,
            # proj weight) are sequenced after on the sync queue so their
            # transfers don't steal HBM bandwidth from xs.
            for ci in range(4):
                nc.sync.dma_start(wv[:, ci, :], wvr[:, ci, :])
                nc.sync.dma_start(xs[:, ci, :], xr[:, ci, :])
            for ci in range(4):
                nc.sync.dma_start(wq[:, ci, :], wqr[:, ci, :])
                nc.sync.dma_start(wk[:, ci, :], wkr[:, ci, :])
            bt = cp.tile([128, 12], f32, tag="bt")
            nc.sync.dma_start(bt[:], btab.ap())
            dm = cp.tile([128, 4, NQ], bf16, tag="dm")
            nc.sync.dma_start(dm[:], dmsk.ap().rearrange("p (a q) -> p a q", a=4))
            wp = cp.tile([128, 4, C], bf16, tag="wp")
            nc.sync.dma_start(wp[:], wp2.ap().rearrange("p (a o) -> p a o", a=4))

            # warmup matmuls: overlap the input DMA phase, get HAM to K=8/8
            wmup = cp.tile([128, 512], bf16, tag="wmup")
            nc.vector.memset(wmup[:], 0.0)
            ones64f = cp.tile([1, 64], f32, tag="ones64f")
            nc.vector.memset(ones64f[:], 1.0)
            wps = pap.tile([128, 512], f32, tag="pa", name="wps")
            for _ in range(16):
                nc.tensor.matmul(wps[:], lhsT=wmup[:, 0:128], rhs=wmup[:],
                                 start=True, stop=True, skip_group_check=True)
            dumt = cp.tile([1, 16], bf16, tag="dumt")
            nc.scalar.activation(dumt[:], wmup[0:1, 0:16], EXP, scale=1.0)

            QT = dp.tile([128, 4, NQ], bf16, tag="QT")
            KT = dp.tile([128, 4, NK], bf16, tag="KT")
            V = dp.tile([128, 10, H, DH + 1], bf16, tag="V")
            OT2 = dp.tile([128, 4, NQ], bf16, tag="OT2")
            oT = dp.tile([128, 4, NQ], bf16, tag="oT")

            # rotating [128, 512] psums for the projection phase
            _qk = [0]

            def qkv_psum():
                n = _qk[0]
                _qk[0] += 1
                if n % 3 == 2:
                    return pap.tile([128, 512], f32, tag="pa", name="qps")
                t = psp.tile([128, 2, 512], f32, tag="ps", name="qps")
                return t[:, n % 2, :]

            # ---- V projection (natural layout) + ones column ----
            nc.vector.memset(V[:, :, :, DH:DH + 1], 1.0)
            for tt in range(10):
                ps = qkv_psum()
                for ci in range(4):
                    nc.tensor.matmul(ps[:], lhsT=xs[:, ci, 128 * tt:128 * (tt + 1)],
                                     rhs=wv[:, ci, :], start=(ci == 0), stop=(ci == 3))
                vdst = V[:, tt, :, 0:DH]
                vsrc = ps[:].rearrange("p (h d) -> p h d", h=H)
                if tt % 2 == 0:
                    nc.scalar.copy(vdst, vsrc)
                else:
                    nc.vector.tensor_copy(vdst, vsrc)

            # ---- normalize chain (for head h), emitted one head late ----
            sden = {}

            def norm_chain(h):
                po, prow = h // 2, slice((h % 2) * 64, (h % 2) * 64 + 64)
                sm, av = sden.pop(h)
                nc.vector.tensor_copy(sm[:], av[0:65, :])
                rs = recp.tile([128, 4], f32, tag="rs", name="rs")
                nc.sync.dma_start(rs[:], sm[64:65, :])
                nc.vector.reciprocal(rs[:], rs[:])
                rcb = recp.tile([64, NQ], f32, tag="rcb", name="rcb")
                nc.sync.dma_start(rcb[0:1, :], rs[:])
                nc.gpsimd.partition_broadcast(rcb[:, :], rcb[0:1, :])
                nc.vector.tensor_mul(OT2[prow, po, :], sm[0:64, :], rcb[:])

            # ---- per-pair Q^T/K^T projection interleaved with attention:
            # pair p+1's projections are emitted between heads 2p and 2p+1 so
            # ScalarE never runs dry of exps at pair boundaries
            def qkt_proj(p):
                ps = qkv_psum()
                for ci in range(4):
                    nc.tensor.matmul(ps[:], lhsT=wq[:, ci, 128 * p:128 * (p + 1)],
                                     rhs=xs[:, ci, KS + F:KS + F + NQ],
                                     start=(ci == 0), stop=(ci == 3))
                nc.vector.tensor_scalar_add(QT[:, p, :], ps[:], bt[:, p:p + 1])
                for t0, t1 in ((0, 512), (512, 1024), (1024, 1280)):
                    ps = qkv_psum()
                    for ci in range(4):
                        nc.tensor.matmul(ps[:, 0:t1 - t0],
                                         lhsT=wk[:, ci, 128 * p:128 * (p + 1)],
                                         rhs=xs[:, ci, t0:t1],
                                         start=(ci == 0), stop=(ci == 3))
                    nc.vector.tensor_scalar_add(KT[:, p, t0:t1], ps[:, 0:t1 - t0],
                                                bt[:, 4 + p:5 + p])

            qkt_proj(0)
            pend = {}

            def s_block(h):
                po = h // 2
                prow = slice((h % 2) * 64, (h % 2) * 64 + 64)
                qh = QT[prow, po, :]

                def kslice(kt):
                    return KT[prow, po, 128 * kt:128 * (kt + 1)]

                # scale keys (k-tiles 0..3): full query range, no mask.
                # two 2-bank psum tiles -> two big exps
                pts = ptp.tile([128, 4, NQ], bf16, tag="pts", name="pts")
                for g in range(2):
                    sc = psp.tile([128, 2, 512], f32, tag="ps", name="sc")
                    for j in range(2):
                        nc.tensor.matmul(sc[:, j, :], lhsT=kslice(2 * g + j),
                                         rhs=qh, start=True, stop=True)
                    nc.scalar.activation(pts[:, 2 * g:2 * g + 2, :], sc[:],
                                         EXP, scale=float(DH) ** -0.5)
                # window keys, merged layout [128, 4, 512]:
                #  slot0: kt4 @ q0:256 | kt5 @ q0:256
                #  slot1: kt6 @ q0:512
                #  slot2: kt7 @ q0:512 (q0:128 is masked to zero)
                #  slot3: kt8 @ q256:512 | kt9 @ q256:512
                pw = ptp.tile([128, 4, NQ], bf16, tag="pw", name="pw")
                wn = psp.tile([128, 2, 512], f32, tag="ps", name="wn")
                nc.tensor.matmul(wn[:, 0, 0:256], lhsT=kslice(4),
                                 rhs=qh[:, 0:256], start=True, stop=True)
                nc.tensor.matmul(wn[:, 0, 256:512], lhsT=kslice(5),
                                 rhs=qh[:, 0:256], start=True, stop=True)
                nc.tensor.matmul(wn[:, 1, :], lhsT=kslice(6),
                                 rhs=qh, start=True, stop=True)
                nc.scalar.activation(pw[:, 0:2, :], wn[:],
                                     EXP, scale=float(DH) ** -0.5)
                wn = psp.tile([128, 2, 512], f32, tag="ps", name="wn")
                nc.tensor.matmul(wn[:, 0, :], lhsT=kslice(7),
                                 rhs=qh, start=True, stop=True)
                nc.tensor.matmul(wn[:, 1, 0:256], lhsT=kslice(8),
                                 rhs=qh[:, 256:512], start=True, stop=True)
                nc.tensor.matmul(wn[:, 1, 256:512], lhsT=kslice(9),
                                 rhs=qh[:, 256:512], start=True, stop=True)
                nc.scalar.activation(pw[:, 2:4, :], wn[:],
                                     EXP, scale=float(DH) ** -0.5)
                nc.vector.tensor_mul(pw[:], pw[:], dm[:])
                pend[h] = (pts, pw)

            def av_block(h):
                pts, pw = pend.pop(h)
                # O^T accumulation (+ sums in row 64 via the ones column)
                av = pap.tile([128, 512], f32, tag="pa", name="av")
                for kt in range(4):
                    nc.tensor.matmul(av[0:65, :], lhsT=V[:, kt, h, :],
                                     rhs=pts[:, kt, :],
                                     start=(kt == 0), stop=False,
                                     skip_group_check=True)
                nc.tensor.matmul(av[0:65, :], lhsT=V[:, 6, h, :],
                                 rhs=pw[:, 1, :],
                                 start=False, stop=False,
                                 skip_group_check=True)
                nc.tensor.matmul(av[0:65, :], lhsT=V[:, 7, h, :],
                                 rhs=pw[:, 2, :],
                                 start=False, stop=False,
                                 skip_group_check=True)
                for j, kt in enumerate((4, 5)):
                    nc.tensor.matmul(av[0:65, 0:256], lhsT=V[:, kt, h, :],
                                     rhs=pw[:, 0, 256 * j:256 * (j + 1)],
                                     start=False, stop=(j == 1),
                                     skip_group_check=True)
                for j, kt in enumerate((8, 9)):
                    nc.tensor.matmul(av[0:65, 256:512], lhsT=V[:, kt, h, :],
                                     rhs=pw[:, 3, 256 * j:256 * (j + 1)],
                                     start=False, stop=(j == 1),
                                     skip_group_check=True)
                sm = recp.tile([65, NQ], f32, tag="sm", name="sm")
                sden[h] = (sm, av)

            # ---- drive the 8 heads, pair p+1's QKT between a pair's heads
            for p in range(4):
                s_block(2 * p)
                if p > 0:
                    norm_chain(2 * p - 1)
                av_block(2 * p)
                if p < 3:
                    qkt_proj(p + 1)
                s_block(2 * p + 1)
                norm_chain(2 * p)
                av_block(2 * p + 1)

            # ---- output projection: head-pair-packed, contract 128 ----
            od = outT.ap().rearrange("(a p) q -> p a q", p=128)
            pjs = []
            for ot in range(4):
                if ot % 2 == 0:
                    pj = pap.tile([128, 512], f32, tag="pa", name="pj")
                else:
                    pj = psp.tile([128, 2, 512], f32, tag="ps", name="pj")[:, 0, :]
                pjs.append(pj)
            # pairs 0..2 depend on already-finished OT2 columns; emit them
            # first so the PE keeps streaming while head 7's chain completes.
            for pr in range(3):
                for ot in range(4):
                    nc.tensor.matmul(pjs[ot][:],
                                     lhsT=wp[:, pr, 128 * ot:128 * (ot + 1)],
                                     rhs=OT2[:, pr, :], start=(pr == 0),
                                     stop=False, skip_group_check=True)
            # head 7's chain with a PE rank-1 broadcast (keeps the PE warm and
            # avoids the gpsimd queue latency right at the tail)
            sm7, av7 = sden.pop(7)
            nc.vector.tensor_copy(sm7[:], av7[0:65, :])
            rs7 = recp.tile([128, 4], f32, tag="rs", name="rs")
            nc.sync.dma_start(rs7[:], sm7[64:65, :])
            nc.vector.reciprocal(rs7[:], rs7[:])
            rr7 = recp.tile([1, NQ], f32, tag="rr7", name="rr7")
            nc.sync.dma_start(rr7[:], rs7[:])
            bp7 = psp.tile([128, 2, 512], f32, tag="ps", name="bp7")[:, 0, :]
            nc.tensor.matmul(bp7[0:64, :], lhsT=ones64f[0:1, :], rhs=rr7[0:1, :],
                             start=True, stop=True, skip_group_check=True)
            nc.vector.tensor_mul(OT2[64:128, 3, :], sm7[0:64, :], bp7[0:64, :])
            for ot in range(4):
                nc.tensor.matmul(pjs[ot][:],
                                 lhsT=wp[:, 3, 128 * ot:128 * (ot + 1)],
                                 rhs=OT2[:, 3, :], start=False,
                                 stop=True, skip_group_check=True)
            for ot in range(4):
                if ot % 2 == 0:
                    nc.scalar.activation(oT[:, ot, :], pjs[ot][:], CPY,
                                         bias=bt[:, 8 + ot:9 + ot], scale=1.0)
                else:
                    nc.vector.tensor_scalar_add(oT[:, ot, :], pjs[ot][:],
                                                bt[:, 8 + ot:9 + ot])
                nc.sync.dma_start(od[:, ot, :], oT[:, ot, :])

        if repeat == 1:
            body()
        else:
            with tc.For_i(0, repeat, 1):
                body()

    nc.compile()
    return nc


def _get_nc(repeat=1):
    key = ("nc", repeat)
    if key not in _CACHE:
        _CACHE[key] = _build(repeat)
    return _CACHE[key]


def _host_prep(x, qkv_w, qkv_b, proj_w, proj_b):
    """Build the 8 per-core input maps."""
    x = np.asarray(x, np.float32).reshape(N, C)
    qkv_w = np.asarray(qkv_w, np.float32)
    qkv_b = np.asarray(qkv_b, np.float32)
    proj_w = np.asarray(proj_w, np.float32)
    proj_b = np.asarray(proj_b, np.float32)

    xs_bf = x.astype(BF16)
    xT_scale = np.ascontiguousarray(xs_bf[0:KS].T)            # [C, 512]
    wqT = np.ascontiguousarray(qkv_w[0:C].T.astype(BF16))
    wkT = np.ascontiguousarray(qkv_w[C:2 * C].T.astype(BF16))
    wvT = np.ascontiguousarray(qkv_w[2 * C:3 * C].T.astype(BF16))
    # head-pair-packed proj weight: wp2[dd, pr*C + c] = proj_w[c, 128*pr + dd]
    wp2 = np.ascontiguousarray(
        proj_w.T.reshape(4, 128, C).transpose(1, 0, 2).reshape(128, 4 * C)
        .astype(BF16))

    # value-bias folds through normalized attention into the proj bias:
    # O = sum_k phat_k (V_k + vb) = O_hat + vb, so out += vb @ proj_w.T
    pb_eff = proj_b + qkv_b[2 * C:3 * C] @ proj_w.T
    btab = np.zeros((128, 12), np.float32)
    for ot in range(4):
        btab[:, ot] = qkv_b[0:C][128 * ot:128 * (ot + 1)]
        btab[:, 4 + ot] = qkv_b[C:2 * C][128 * ot:128 * (ot + 1)]
        btab[:, 8 + ot] = pb_eff[128 * ot:128 * (ot + 1)]

    # tril01[j, q] = 1 if key j <= query q (within the same frame)
    tril01 = (np.arange(F)[:, None] <= np.arange(F)[None, :])

    in_maps = []
    for i in range(NCORES):
        win = np.zeros((KW, C), BF16)
        lo = F * (2 * i - 1)
        src = xs_bf[max(0, lo):F * (2 * i + 2)]
        win[KW - len(src):] = src
        xTi = np.empty((C, NK), BF16)
        xTi[:, 0:KS] = xT_scale
        xTi[:, KS:] = win.T

        vf = 1.0 if (2 * i - 1) >= 2 else 0.0
        vd = np.array([1.0 if (2 * i) >= 2 else 0.0,
                       1.0 if (2 * i + 1) >= 2 else 0.0], np.float32)
        # mask image, [128, 4, 512]:
        #  slot0: kt4 flag | kt5 flag (both only live for q0:256)
        #  slot1 (kt6): tril*vd0 for q0:256, vd0 for q256:512
        #  slot2 (kt7): 0 for q0:128, tril*vd0 for q128:256, vd0 for q256:512
        #  slot3 (kt8, kt9): tril halves * vd1 (queries q256:512)
        dmsk = np.zeros((128, 4, NQ), np.float32)
        dmsk[:, 0, :] = vf
        dmsk[:, 1, 0:256] = tril01[0:128, :] * vd[0]
        dmsk[:, 1, 256:512] = vd[0]
        dmsk[:, 2, 0:128] = 0.0
        dmsk[:, 2, 128:256] = tril01[128:256, 128:256] * vd[0]
        dmsk[:, 2, 256:512] = vd[0]
        dmsk[:, 3, 0:256] = tril01[0:128, :] * vd[1]
        dmsk[:, 3, 256:512] = tril01[128:256, :] * vd[1]
        in_maps.append({
            "xT": xTi, "wqT": wqT, "wkT": wkT, "wvT": wvT, "wp2": wp2,
            "btab": btab,
            "dmsk": dmsk.reshape(128, 4 * NQ).astype(BF16),
        })
    return in_maps


def _check_fixed_params(block_mask, video_mask, frame_seqlen,
                        sliding_window_size, num_frame_per_block,
                        num_frame_for_scale):
    if int(frame_seqlen) != F or int(sliding_window_size) != 2 \
            or int(num_frame_per_block) != 1 or int(num_frame_for_scale) != 2:
        return False
    vm = np.asarray(video_mask)
    if not bool(vm.all()):
        return False
    bm = np.asarray(block_mask)
    if bm.shape != (N, N):
        return False
    # spot-check causality structure of block_mask (full check is 16M bools)
    idx = np.linspace(0, N - 1, 64).astype(int)
    sub = bm[np.ix_(idx, idx)]
    if not np.array_equal(sub, np.tril(np.ones_like(sub))):
        return False
    return True


def _numpy_reference(x, block_mask, video_mask, qkv_w, qkv_b, proj_w, proj_b,
                     frame_seqlen, sliding_window_size, num_frame_per_block,
                     num_frame_for_scale):
    """Fallback: direct numpy evaluation of the reference semantics."""
    x = np.asarray(x, np.float32)
    b, n, c = x.shape
    dh = c // H
    qkv = (x @ np.asarray(qkv_w).T + np.asarray(qkv_b)).reshape(b, n, 3, H, dh)
    qkv = qkv.transpose(2, 0, 3, 1, 4)
    q, k, v = qkv[0], qkv[1], qkv[2]
    mask = np.asarray(block_mask)[:n, :n][None, None]
    vm = np.asarray(video_mask)[:, None, None, None]
    mask = mask | ~vm
    fs = int(frame_seqlen)
    if int(sliding_window_size) > 0 and fs is not None:
        f = np.arange(n) // fs
        w = int(sliding_window_size) * int(num_frame_per_block)
        sliding = (f[None, :] <= f[:, None]) & (f[None, :] >= f[:, None] - w + 1)
        mask = mask & sliding[None, None]
        if int(num_frame_for_scale) > 0:
            s = int(num_frame_for_scale) * fs
            mask = mask.copy()
            mask[:, :, :, :s] = True
    scores = np.einsum('bhqd,bhkd->bhqk', q, k) * (dh ** -0.5)
    scores = np.where(mask, scores, np.float32(-1e30))
    scores -= scores.max(axis=-1, keepdims=True)
    e = np.exp(scores)
    attn = e / e.sum(axis=-1, keepdims=True)
    o = np.einsum('bhqk,bhkd->bhqd', attn, v)
    o = o.transpose(0, 2, 1, 3).reshape(b, n, c)
    return (o @ np.asarray(proj_w).T + np.asarray(proj_b)).astype(np.float32)


def kernel(x, block_mask, video_mask, qkv_w, qkv_b, proj_w, proj_b,
           frame_seqlen, sliding_window_size, num_frame_per_block,
           num_frame_for_scale):
    if not _check_fixed_params(block_mask, video_mask, frame_seqlen,
                               sliding_window_size, num_frame_per_block,
                               num_frame_for_scale):
        return _numpy_reference(x, block_mask, video_mask, qkv_w, qkv_b,
                                proj_w, proj_b, frame_seqlen,
                                sliding_window_size, num_frame_per_block,
                                num_frame_for_scale)

    from concourse.bass_utils import run_bass_kernel_spmd

    nc = _get_nc()
    in_maps = _host_prep(x, qkv_w, qkv_b, proj_w, proj_b)
    res = run_bass_kernel_spmd(nc, in_maps, core_ids=list(range(NCORES)))
    out = np.empty((N, C), np.float32)
    for i in range(NCORES):
        out[NQ * i:NQ * (i + 1)] = res.results[i]["outT"].T
    return out.reshape(1, N, C)
